# revision 6
# baseline (speedup 1.0000x reference)
"""Trainium2 Bass kernel for nn_CompositionBlock (gnn_message_passing).

Data-parallel over batch B=8 across 8 NeuronCores (one sample per core).

U-first bilinear formulation: for each token j the bilinears are
  tde[p,j] = sum_{t,d} W1[p,t,d] * (tok[j,t]*dep[j,d])
  comp[o,j] = sum_{t,p} W2[o,t,p] * (tok[j,t]*h[p,j])
The elementwise outer-product operands U/V are built by DVE from
host-replicated f16 tiles (all-SBUF 16-bit => DVE fast mode), and each
bilinear is ONE PSUM accumulation chain of K-chunk matmuls (no selection
-matrix reduction matmuls).  h replication across partitions is done on
the PE with a selection matrix.  Head-scatter stays a one-hot matmul.
"""

import json

import numpy as np

B, S, T, D, P = 8, 256, 128, 64, 128
NCORES = 8
JT = S // 128  # token tiles (j) per core


# ----------------------------------------------------------------------------
# Compat: the walrus build in this container accepts at most one sync-wait on
# CTRL-class instructions, but TileContext's tail drain packs several. Split
# any multi-wait instruction into a chain of single-wait clones.
# ----------------------------------------------------------------------------
def _split_multiwait_bir(bir_json_bytes: bytes) -> bytes:
    bir = json.loads(bir_json_bytes)
    for func in bir.get("functions", []):
        for bb in func.get("blocks", []):
            new_instructions = []
            for ins in bb.get("instructions", []):
                si = ins.get("sync_info") or {}
                waits = si.get("on_wait") or []
                if len(waits) > 1:
                    for i, w in enumerate(waits[:-1]):
                        new_instructions.append({
                            "debug": ins.get("debug", 0),
                            "engine": ins["engine"],
                            "ins": [],
                            "name": f"{ins['name']}_w{i}",
                            "opcode": "NoOp",
                            "outs": [],
                            "sync_info": {"on_wait": [w], "on_update": []},
                        })
                    ins["sync_info"] = {
                        "on_wait": [waits[-1]],
                        "on_update": si.get("on_update") or [],
                    }
                new_instructions.append(ins)
            bb["instructions"] = new_instructions
    return json.dumps(bir).encode()


def _install_compat():
    import concourse.bass_utils as bu

    if getattr(bu.compile_bir_kernel, "_multiwait_patched", False):
        return
    orig = bu.compile_bir_kernel

    def patched(bir_json, tmpdir, neff_name="file.neff"):
        return orig(_split_multiwait_bir(bir_json), tmpdir, neff_name)

    patched._multiwait_patched = True
    bu.compile_bir_kernel = patched
    try:
        import concourse.bass2jax as b2j

        if getattr(b2j, "compile_bir_kernel", None) is not None:
            b2j.compile_bir_kernel = patched
    except ImportError:
        pass


_NC_CACHE = {}


def build_nc():
    if "nc" in _NC_CACHE:
        return _NC_CACHE["nc"]
    import concourse.bass as bass
    import concourse.tile as tile
    from concourse import mybir
    from concourse.masks import make_identity

    f32 = mybir.dt.float32
    f16 = mybir.dt.float16
    Alu = mybir.AluOpType
    Act = mybir.ActivationFunctionType

    nc = bass.Bass(trn_type="TRN2")

    # DRAM inputs (all host-prepped into final SBUF layouts)
    tok16_d = nc.dram_tensor("tok16", [4, 128, 1024], f16, kind="ExternalInput")
    dep8_d = nc.dram_tensor("dep8", [128, 1024], f16, kind="ExternalInput")
    w1_d = nc.dram_tensor("w1", [4, 128, 2048], f16, kind="ExternalInput")
    w2_d = nc.dram_tensor("w2", [8, 128, 2048], f16, kind="ExternalInput")
    rep_d = nc.dram_tensor("rep", [128, 1024], f16, kind="ExternalInput")
    iota_d = nc.dram_tensor("iota", [1, S], f16, kind="ExternalInput")
    headsf_d = nc.dram_tensor("headsf", [JT, 128, 1], f32, kind="ExternalInput")
    wr_d = nc.dram_tensor("wr", [JT, 128, 1], f32, kind="ExternalInput")
    bdep_d = nc.dram_tensor("bdep", [128, 1], f32, kind="ExternalInput")
    bcomp_d = nc.dram_tensor("bcomp", [128, 1], f32, kind="ExternalInput")
    base_d = nc.dram_tensor("base", [128, 1], f32, kind="ExternalInput")
    c0_d = nc.dram_tensor("c0", [1, T], f32, kind="ExternalInput")
    out_d = nc.dram_tensor("out", [S, T], f32, kind="ExternalOutput")

    def bcast_row(dram_ap):
        return bass.AP(
            tensor=dram_ap.tensor,
            offset=dram_ap.offset,
            ap=[[0, 128]] + list(dram_ap.ap[1:]),
        )

    with tile.TileContext(nc) as tc:
        with (
            tc.tile_pool(name="consts", bufs=1) as consts,
            tc.tile_pool(name="weights", bufs=1) as weights,
            tc.tile_pool(name="acts", bufs=1) as acts,
            tc.tile_pool(name="h8p", bufs=3) as h8p,
            tc.tile_pool(name="vp", bufs=3) as vp,
            tc.tile_pool(name="work", bufs=2) as work,
            tc.tile_pool(name="pstde", bufs=1, space="PSUM") as pstde,
            tc.tile_pool(name="psrep", bufs=2, space="PSUM") as psrep,
            tc.tile_pool(name="pscomp", bufs=1, space="PSUM") as pscomp,
            tc.tile_pool(name="psmm", bufs=2, space="PSUM") as psmm,
            tc.tile_pool(name="psfin", bufs=1, space="PSUM") as psfin,
        ):
            # ---- small consts (issue first; they are tiny) ----
            ident16 = consts.tile([128, 128], f16)
            make_identity(nc, ident16)

            iota_b = consts.tile([128, S], f16)
            nc.sync.dma_start(out=iota_b, in_=bcast_row(iota_d[:, :]))
            c0_b = consts.tile([128, T], f32)
            nc.sync.dma_start(out=c0_b, in_=bcast_row(c0_d[:, :]))
            bdep_c = consts.tile([128, 1], f32)
            nc.sync.dma_start(out=bdep_c, in_=bdep_d[:, :])
            bcomp_c = consts.tile([128, 1], f32)
            nc.sync.dma_start(out=bcomp_c, in_=bcomp_d[:, :])
            base_c = consts.tile([128, 1], f32)
            nc.sync.dma_start(out=base_c, in_=base_d[:, :])
            headsf_t = []
            wr_t = []
            for jt in range(JT):
                ht = consts.tile([128, 1], f32, name=f"hf{jt}", tag=f"hf{jt}")
                nc.sync.dma_start(out=ht, in_=headsf_d[jt, :, :])
                headsf_t.append(ht)
                wt = consts.tile([128, 1], f32, name=f"wrt{jt}", tag=f"wrt{jt}")
                nc.sync.dma_start(out=wt, in_=wr_d[jt, :, :])
                wr_t.append(wt)

            # ---- big DMAs, in consumption order, spread across queues ----
            tok16_sb = []
            for q in range(4):
                t = weights.tile([128, 1024], f16, name=f"tok16_{q}", tag=f"tok16_{q}")
                eng = [nc.sync, nc.scalar, nc.gpsimd, nc.sync][q]
                eng.dma_start(out=t, in_=tok16_d[q, :, :])
                tok16_sb.append(t)
            dep8_sb = weights.tile([128, 1024], f16)
            nc.scalar.dma_start(out=dep8_sb, in_=dep8_d[:, :])
            w1_sb = []
            for q in range(4):
                t = weights.tile([128, 2048], f16, name=f"w1_{q}", tag=f"w1_{q}")
                eng = [nc.sync, nc.scalar, nc.gpsimd, nc.sync][q]
                eng.dma_start(out=t, in_=w1_d[q, :, :])
                w1_sb.append(t)
            rep_sb = weights.tile([128, 1024], f16)
            nc.gpsimd.dma_start(out=rep_sb, in_=rep_d[:, :])
            w2_sb = []
            for pb in range(8):
                t = weights.tile([128, 2048], f16, name=f"w2_{pb}", tag=f"w2_{pb}")
                eng = [nc.sync, nc.scalar, nc.gpsimd][pb % 3]
                eng.dma_start(out=t, in_=w2_d[pb, :, :])
                w2_sb.append(t)

            # ---- soh tiles early (DVE is idle during stage-1 rampup) ----
            # soh[jt][j, i] = (head[j] == i) * wr[j], f16
            soh = []
            for jt in range(JT):
                s = acts.tile([128, S], f16, name=f"soh{jt}", tag=f"soh{jt}")
                nc.vector.tensor_scalar(
                    out=s, in0=iota_b, scalar1=headsf_t[jt], scalar2=wr_t[jt],
                    op0=Alu.is_equal, op1=Alu.mult,
                )
                soh.append(s)

            # ---- stage 1: tde[p, j] = sum over 64 K-chunks ----
            # chunk order ci1 = q*16 + db*4 + ti   (tb = 4q + ti)
            U_sb = [
                acts.tile([128, 4096], f16, name=f"U{db}", tag=f"U{db}")
                for db in range(4)
            ]
            tde_ps = pstde.tile([128, S], f32)
            ci1 = 0
            for q in range(4):
                for db in range(4):
                    # U[db][:, q*1024:(q+1)*1024] = tok16_q * bcast(dep8[:, db])
                    out_v = U_sb[db][:, q * 1024:(q + 1) * 1024].rearrange(
                        "p (tb j) -> p tb j", j=256)
                    in0_v = tok16_sb[q][:, :].rearrange("p (tb j) -> p tb j", j=256)
                    in1_v = dep8_sb[:, db * 256:(db + 1) * 256].unsqueeze(1) \
                        .broadcast_to([128, 4, 256])
                    nc.vector.tensor_tensor(out=out_v, in0=in0_v, in1=in1_v,
                                            op=Alu.mult)
                    for ti in range(4):
                        nc.tensor.matmul(
                            tde_ps,
                            w1_sb[q][:, (db * 4 + ti) * 128:(db * 4 + ti + 1) * 128],
                            U_sb[db][:, (q * 4 + ti) * 256:(q * 4 + ti + 1) * 256],
                            start=(ci1 == 0),
                            stop=(ci1 == 63),
                        )
                        ci1 += 1

            # h = tanh(tde + b_dep), f16 [p, j]
            hT = acts.tile([128, S], f16)
            nc.scalar.activation(hT, tde_ps, Act.Tanh, bias=bdep_c)

            # ---- stage 2: comp[o, j] = sum over 128 K-chunks ----
            # chunk order ci2 = pb*16 + tb
            comp_ps = pscomp.tile([128, S], f32)

            rep_ps = {}

            def emit_rep(pb):
                ps = psrep.tile([128, S], f32, name="rep_ps", tag="rep_ps")
                nc.tensor.matmul(
                    ps, rep_sb[:, pb * 128:(pb + 1) * 128], hT,
                    start=True, stop=True,
                )
                rep_ps[pb] = ps

            emit_rep(0)
            for pb in range(8):
                if pb + 1 < 8:
                    emit_rep(pb + 1)
                h8 = h8p.tile([128, S], f16, name="h8", tag="h8")
                nc.scalar.copy(h8, rep_ps.pop(pb))
                for q in range(4):
                    v = vp.tile([128, 1024], f16, name="V", tag="V")
                    out_v = v[:, :].rearrange("p (tb j) -> p tb j", j=256)
                    in0_v = tok16_sb[q][:, :].rearrange("p (tb j) -> p tb j", j=256)
                    in1_v = h8[:, :].unsqueeze(1).broadcast_to([128, 4, 256])
                    nc.vector.tensor_tensor(out=out_v, in0=in0_v, in1=in1_v,
                                            op=Alu.mult)
                    for ti in range(4):
                        tb = 4 * q + ti
                        ci2 = pb * 16 + tb
                        nc.tensor.matmul(
                            comp_ps,
                            w2_sb[pb][:, tb * 128:(tb + 1) * 128],
                            v[:, ti * 256:(ti + 1) * 256],
                            start=(ci2 == 0),
                            stop=(ci2 == 127),
                        )

            # spec = tanh(comp + b_comp); delta = spec - base  (f16, [o, j])
            specT = work.tile([128, S], f32, name="specT", tag="specT")
            nc.scalar.activation(specT, comp_ps, Act.Tanh, bias=bcomp_c)
            deltaT = acts.tile([128, S], f16)
            nc.vector.tensor_scalar(
                out=deltaT, in0=specT, scalar1=base_c, scalar2=None,
                op0=Alu.subtract,
            )

            # transpose deltaT -> delta[j, o] per token tile
            delta_sb = []
            for jt in range(JT):
                dps = psmm.tile([128, 128], f16, name="dps", tag="dps")
                nc.tensor.transpose(
                    dps, deltaT[:, jt * 128:(jt + 1) * 128], ident16
                )
                dsb = acts.tile([128, 128], f16, name=f"delta{jt}", tag=f"delta{jt}")
                nc.scalar.copy(dsb, dps)
                delta_sb.append(dsb)

            # fin[i, o] = sum_jt soh[jt][:, i-chunk].T @ delta[jt]
            fin_ps = psfin.tile([128, S], f32)
            for ic in range(2):
                for jt in range(JT):
                    nc.tensor.matmul(
                        fin_ps[:, ic * 128:(ic + 1) * 128],
                        soh[jt][:, ic * 128:(ic + 1) * 128],
                        delta_sb[jt],
                        start=(jt == 0),
                        stop=(jt == JT - 1),
                    )
            for ic in range(2):
                outsb = work.tile([128, T], f32, name="outsb", tag="outsb")
                nc.vector.tensor_add(
                    outsb, fin_ps[:, ic * 128:(ic + 1) * 128], c0_b
                )
                nc.sync.dma_start(
                    out=out_d[ic * 128:(ic + 1) * 128, :], in_=outsb
                )

    _NC_CACHE["nc"] = nc
    return nc


def prep_core_inputs(token_embeddings, dep_embeddings, dep_heads,
                     W_dep, b_dep, W_comp, b_comp, W_red, b_red):
    f32 = np.float32
    f16 = np.float16
    tok = np.asarray(token_embeddings, dtype=f32)
    dep = np.asarray(dep_embeddings, dtype=f32)
    heads = np.asarray(dep_heads)
    W_dep = np.asarray(W_dep, dtype=f32)
    b_dep = np.asarray(b_dep, dtype=f32)
    W_comp = np.asarray(W_comp, dtype=f32)
    b_comp = np.asarray(b_comp, dtype=f32)
    wr = np.asarray(W_red, dtype=f32)[0]
    b_red = np.asarray(b_red, dtype=f32)

    # W1sb[(d',t'), (q, db, ti, p)] = W_dep[p, 8*(4q+ti)+t', 16*db+d']
    A = W_dep.reshape(P, 4, 4, 8, 4, 16)          # [p, q, ti, t', db, d']
    w1 = np.ascontiguousarray(
        A.transpose(5, 3, 1, 4, 2, 0).reshape(128, 4, 2048).transpose(1, 0, 2)
    ).astype(f16)                                 # [q, 128, 2048]

    # W2sb[(p',t'), (pb, tb, o)] = W_comp[o, 8*tb+t', 16*pb+p']
    Bm = W_comp.reshape(T, 16, 8, 8, 16)          # [o, tb, t', pb, p']
    w2 = np.ascontiguousarray(
        Bm.transpose(4, 2, 3, 1, 0).reshape(128, 8, 2048).transpose(1, 0, 2)
    ).astype(f16)                                 # [pb, 128, 2048]

    # rep[k, (pb, r)] = 1 if k == 16*pb + r//8
    rep = np.zeros((128, 8, 128), dtype=f16)
    r_ = np.arange(128)
    for pb in range(8):
        rep[16 * pb + r_ // 8, pb, r_] = 1.0
    rep = rep.reshape(128, 1024)

    base = np.tanh(b_comp)
    c0 = (base * wr.sum() + b_red[0]).astype(f32)
    iota = np.arange(S, dtype=f16).reshape(1, S)
    headsf = heads.astype(f32).reshape(B, JT, 128, 1)
    wr_t = np.ascontiguousarray(wr.reshape(JT, 128, 1))

    shared = {
        "w1": w1, "w2": w2, "rep": rep,
        "iota": iota, "wr": wr_t,
        "bdep": b_dep.reshape(128, 1),
        "bcomp": b_comp.reshape(128, 1),
        "base": base.reshape(128, 1).astype(f32),
        "c0": c0.reshape(1, T),
    }
    in_maps = []
    for c in range(NCORES):
        # tok16[(rep16, t'), (tb, j)] = tok[c][j, 8*tb + t']
        tokT3 = np.ascontiguousarray(tok[c].T).reshape(16, 8, S)   # [tb, t', j]
        tmp = tokT3.transpose(1, 0, 2)                             # [t', tb, j]
        tok16 = np.ascontiguousarray(
            np.broadcast_to(tmp[None], (16, 8, 16, S))
            .reshape(128, 4, 1024).transpose(1, 0, 2)
        ).astype(f16)                                              # [q, 128, 1024]
        # dep8[(d', rep8), (db, j)] = dep[c][j, 16*db + d']
        depT3 = np.ascontiguousarray(dep[c].T).reshape(4, 16, S)   # [db, d', j]
        dmp = depT3.transpose(1, 0, 2)                             # [d', db, j]
        dep8 = np.ascontiguousarray(
            np.broadcast_to(dmp[:, None], (16, 8, 4, S)).reshape(128, 4 * S)
        ).astype(f16)
        m = dict(shared)
        m["tok16"] = tok16
        m["dep8"] = dep8
        m["headsf"] = np.ascontiguousarray(headsf[c])
        in_maps.append(m)
    return in_maps


def kernel(**inputs) -> np.ndarray:
    _install_compat()
    from concourse.bass_utils import run_bass_kernel_spmd

    nc = build_nc()
    in_maps = prep_core_inputs(**inputs)
    res = run_bass_kernel_spmd(nc, in_maps, core_ids=list(range(NCORES)))
    out = np.stack([res.results[c]["out"] for c in range(NCORES)], axis=0)
    return out.astype(np.float32)


# aliases used by test harness
_build_nc = build_nc
_prep_core_inputs = prep_core_inputs
